# revision 1
# baseline (speedup 1.0000x reference)
"""Trainium2 Bass kernel for iterated rule application (differentiable ILP).

Math (see reference):
    A = softmax(rule_weights[:, 0, :], axis=-1)            # [P, P]
    repeat n_iterations times:
        derived = A @ facts                                # [P, C]
        facts   = max(facts, derived.max(axis=0))          # row-max bcast over rows

Distribution: facts is sharded along the C (constants) axis across the 8
NeuronCores (column parallel); rule_weights is replicated.  Every step is
column-local, so there is no inter-core communication at all.

Per-core structure ("layout-2": facts is the matmul's stationary operand):
  Fh [p, c] stays resident in SBUF as [128, 4*8192] fp16 (p-chunk major) -
  fp16 matmuls run the PE at full rate and the inputs carry ~2^-11 relative
  quantization, comparable to TF32.  For each c-chunk j (128 cols):
  PSUM[c, r] += Fh[:, j]^T @ A^T over the 4 p-chunks; a free-axis reduce_max
  over r yields m[c] in fp32 on c-partitions.  Chunks are processed in blocks
  of 4 (512 cols): the four m columns [128, 4] are PE-transposed to [4, 128],
  evicted by ScalarE (casting to fp16), and broadcast to all 128 partitions
  with one-hot-weight matmuls into a [128, 512] PSUM bank.  The Fh update
  max(Fh, bcast) is split between VectorE and GpSimd (both reading a ScalarE
  SBUF copy, in fp16 at DVE 2x rate).

  Monotonicity (m_1 <= m_2 <= ... <= m_n) makes the final output simply
  max(initial_facts, m_n).  The last iteration therefore re-streams the
  exact fp32 facts from HBM, broadcasts m_n in full fp32 (plain fp32
  matmuls), combines, and streams the result out - so every element of the
  output that comes from initial_facts is bit-exact, and m-driven elements
  carry only the inherent fp16-input matmul error.
"""

import functools

import numpy as np

import concourse.bacc as bacc
import concourse.mybir as mybir
from concourse.tile import TileContext
from concourse.bass_utils import run_bass_kernel_spmd
from concourse.masks import make_identity

P = 512            # n_predicates
C = 65536          # n_constants
NCORES = 8
CS = C // NCORES   # columns per core
PS = P // 128      # p-chunks
F32 = mybir.dt.float32
F16 = mybir.dt.float16
X = mybir.AxisListType.X
MAX = mybir.AluOpType.max


def build(n_iters: int, cs: int = CS, paired: bool = False):
    """Build + compile the per-core program.  `cs` is the per-core column
    count (overridable for fast simulator checks)."""
    nch = cs // 128      # c-chunks
    nblk = cs // 512     # 4-chunk blocks
    nload = max(1, cs // 2048)
    ldw = cs // nload

    nc = bacc.Bacc("TRN2", target_bir_lowering=False, debug=False,
                   num_devices=NCORES)
    facts_in = nc.dram_tensor("facts", [P, cs], F32, kind="ExternalInput").ap()
    rw = nc.dram_tensor("rw", [P, P], F32, kind="ExternalInput").ap()
    oneh_in = nc.dram_tensor("onehots", [4, 512], F32, kind="ExternalInput").ap()
    facts_out = nc.dram_tensor("out", [P, cs], F32, kind="ExternalOutput").ap()

    with TileContext(nc) as tc:
        with (
            tc.tile_pool(name="const", bufs=1) as cpool,
            tc.tile_pool(name="work", bufs=3) as wpool,
            tc.tile_pool(name="ld", bufs=6) as ldpool,
            tc.tile_pool(name="accp", bufs=2, space="PSUM") as acc_pool,
            tc.tile_pool(name="bcp", bufs=3, space="PSUM") as bc_pool,
            tc.tile_pool(name="tpp", bufs=1, space="PSUM") as tp_pool,
        ):
            ident = cpool.tile([128, 128], F32, tag="ident")
            make_identity(nc, ident[:, :])
            # onehots[k, k*128:(k+1)*128] = 1, else 0 (broadcast weights)
            oneh32 = cpool.tile([4, 512], F32, tag="oneh32")
            nc.sync.dma_start(out=oneh32[:, :], in_=oneh_in[:, :])
            oneh16 = cpool.tile([4, 512], F16, tag="oneh16")
            nc.vector.tensor_copy(oneh16[:, :], oneh32[:, :])

            # ---- A = softmax(rw, axis=-1), stored transposed in fp16:
            # AT[:, s*512 + r] = A[r, s*128 + p_local]
            AT = cpool.tile([128, PS * P], F16, tag="AT")
            for t in range(PS):               # r-chunk
                rwt = wpool.tile([128, P], F32, tag="rwt")
                nc.sync.dma_start(out=rwt[:, :], in_=rw[t * 128:(t + 1) * 128, :])
                negmax = wpool.tile([128, 1], F32, tag="negmax")
                nc.vector.tensor_reduce(negmax[:, :], rwt[:, :], axis=X, op=MAX,
                                        negate=True)
                ex = wpool.tile([128, P], F32, tag="ex")
                nc.scalar.activation(ex[:, :], rwt[:, :],
                                     mybir.ActivationFunctionType.Exp,
                                     bias=negmax[:, 0:1], scale=1.0)
                ssum = wpool.tile([128, 1], F32, tag="ssum")
                nc.vector.reduce_sum(ssum[:, :], ex[:, :], axis=X)
                rinv = wpool.tile([128, 1], F32, tag="rinv")
                nc.vector.reciprocal(rinv[:, :], ssum[:, :])
                at = wpool.tile([128, P], F32, tag="at")
                nc.vector.tensor_scalar_mul(at[:, :], ex[:, :], rinv[:, 0:1])
                for s in range(PS):           # p-chunk: transpose 128x128 block
                    pt = tp_pool.tile([128, 128], F32, tag="tp")
                    nc.tensor.transpose(pt[:, :], at[:, s * 128:(s + 1) * 128],
                                        ident[:, :])
                    nc.vector.tensor_copy(
                        AT[:, s * P + t * 128: s * P + (t + 1) * 128], pt[:, :])

            # ---- facts resident in SBUF (fp16): Fh[:, s*cs + c]
            # (gpsimd software-DGE DMA casts fp32->fp16 inline)
            Fh = cpool.tile([128, PS * cs], F16, tag="Fh")
            for s in range(PS):
                for g4 in range(nload):
                    nc.gpsimd.dma_start(
                        out=Fh[:, s * cs + g4 * ldw: s * cs + (g4 + 1) * ldw],
                        in_=facts_in[s * 128:(s + 1) * 128,
                                     g4 * ldw:(g4 + 1) * ldw])

            # ---- iterations (block-local; no global barrier)
            for it in range(n_iters):
                last = it == n_iters - 1
                for b in range(nblk):
                    mblk = wpool.tile([128, 4], F32, tag="mblk")
                    if paired:
                        for h in range(2):    # pairs of c-chunks
                            acc2 = acc_pool.tile([128, 1024], F32, tag="acc2")
                            for q in range(2):
                                j = 4 * b + 2 * h + q
                                for s in range(PS):
                                    nc.tensor.matmul(
                                        acc2[:, q * 512:(q + 1) * 512],
                                        Fh[:, s * cs + j * 128:
                                           s * cs + (j + 1) * 128],
                                        AT[:, s * P:(s + 1) * P],
                                        start=(s == 0), stop=(s == PS - 1))
                            nc.vector.tensor_reduce(
                                mblk[:, 2 * h:2 * h + 2],
                                acc2[:, :].rearrange("p (a b) -> p a b", b=512),
                                axis=X, op=MAX)
                    else:
                        for q in range(4):
                            j = 4 * b + q
                            acc = acc_pool.tile([128, 512], F32, tag="acc2")
                            for s in range(PS):
                                nc.tensor.matmul(
                                    acc[:, :],
                                    Fh[:, s * cs + j * 128:
                                       s * cs + (j + 1) * 128],
                                    AT[:, s * P:(s + 1) * P],
                                    start=(s == 0), stop=(s == PS - 1))
                            nc.vector.tensor_reduce(mblk[:, q:q + 1],
                                                    acc[:, :], axis=X, op=MAX)

                    # m-path for this block: transpose -> evict -> broadcast
                    pt = tp_pool.tile([4, 128], F32, tag="tp")
                    nc.tensor.transpose(pt[:, :], mblk[:, :], ident[:, :])
                    bc = bc_pool.tile([128, 512], F32, tag="bc")
                    if not last:
                        tt16 = wpool.tile([4, 128], F16, tag="tt16")
                        nc.scalar.copy(tt16[:, :], pt[:, :])
                        for q in range(4):
                            nc.tensor.matmul(
                                bc[:, q * 128:(q + 1) * 128],
                                oneh16[:, q * 128:(q + 1) * 128],
                                tt16[:, :], start=True, stop=True)
                        bcs = wpool.tile([128, 512], F16, tag="bcs")
                        nc.scalar.copy(bcs[:, :], bc[:, :])
                        for s in range(PS):
                            fsl = Fh[:, s * cs + b * 512: s * cs + (b + 1) * 512]
                            nc.vector.tensor_max(fsl, fsl, bcs[:, :])
                    else:
                        # exact fp32 broadcast of m_n, combine with exact
                        # re-streamed initial facts, stream out
                        tt32 = wpool.tile([4, 128], F32, tag="tt32")
                        nc.scalar.copy(tt32[:, :], pt[:, :])
                        for q in range(4):
                            nc.tensor.matmul(
                                bc[:, q * 128:(q + 1) * 128],
                                oneh32[:, q * 128:(q + 1) * 128],
                                tt32[:, :], start=True, stop=True)
                        bcs32 = wpool.tile([128, 512], F32, tag="bcs32")
                        nc.scalar.copy(bcs32[:, :], bc[:, :])
                        for s in range(PS):
                            l = ldpool.tile([128, 512], F32, tag="lo")
                            nc.sync.dma_start(
                                out=l[:, :],
                                in_=facts_in[s * 128:(s + 1) * 128,
                                             b * 512:(b + 1) * 512])
                            nc.vector.tensor_max(l[:, :], l[:, :],
                                                 bcs32[:, :])
                            nc.sync.dma_start(
                                out=facts_out[s * 128:(s + 1) * 128,
                                              b * 512:(b + 1) * 512],
                                in_=l[:, :])
            if n_iters == 0:
                for s in range(PS):
                    for g4 in range(nload):
                        l = ldpool.tile([128, ldw], F32, tag="l")
                        nc.sync.dma_start(
                            out=l[:, :],
                            in_=facts_in[s * 128:(s + 1) * 128,
                                         g4 * ldw:(g4 + 1) * ldw])
                        nc.sync.dma_start(
                            out=facts_out[s * 128:(s + 1) * 128,
                                          g4 * ldw:(g4 + 1) * ldw],
                            in_=l[:, :])

    nc.compile()
    return nc


def _onehots() -> np.ndarray:
    return np.kron(np.eye(4, dtype=np.float32),
                   np.ones((1, 128), dtype=np.float32)).reshape(4, 512)


@functools.lru_cache(maxsize=4)
def _built(n_iters: int):
    return build(n_iters)


def kernel(initial_facts, rule_weights, n_iterations):
    n = int(n_iterations)
    f = np.ascontiguousarray(np.asarray(initial_facts, dtype=np.float32))
    rwm = np.ascontiguousarray(
        np.asarray(rule_weights, dtype=np.float32).reshape(P, P))
    assert f.shape == (P, C), f.shape

    nc = _built(n)
    oneh = _onehots()
    in_maps = [
        {"facts": np.ascontiguousarray(f[:, k * CS:(k + 1) * CS]),
         "rw": rwm, "onehots": oneh}
        for k in range(NCORES)
    ]
    res = run_bass_kernel_spmd(nc, in_maps, list(range(NCORES)))
    return np.concatenate([res.results[k]["out"] for k in range(NCORES)],
                          axis=1)

